# revision 19
# baseline (speedup 1.0000x reference)
"""Trainium2 Bass kernel for sliding-window attention (nn_AttentionBase).

Reference computation (T=2048, D=2048, H=16, HD=128, WINDOW=512):
  q/k/v = hidden @ wq/wk/wv ; q,k: per-head RMSNorm + RoPE
  scores = q k^T / sqrt(HD), causal sliding-window mask (t-s in [0, 512))
  out = softmax(scores) @ v @ wo

Sharding: tensor-parallel over heads, 2 heads per core on 8 cores; each core
computes a partial out = ctx_heads @ wo_heads; host sums the 8 partials.

Device-side design notes:
- All big matmuls run as float32r (full PE rate at free-dim >= 256).
- hidden is fed pre-transposed (hT), so Q/K come out in [head_dim, t] layout
  directly, K never needs a transpose for scores = K^T_tile.T @ Q.
- The head_dim axis of wq/wk is pre-permuted (even dims | odd dims) so RoPE
  pairs become partition blocks [0:64) and [64:128).
- Softmax runs without max-subtraction (|scaled scores| <= sqrt(HD) ~ 11.3,
  exp stays finite in fp32); the mask is a 0/1 multiply after exp; the
  denominator comes from a ones-column matmul, its reciprocal from
  exp(-ln(den)) on ACT (keeps every ACT op in the exp/ln table set), and is
  partition-broadcast with a K=1 matmul.
"""

import sys

import numpy as np

if "/opt/trn_rl_repo" not in sys.path:
    sys.path.insert(0, "/opt/trn_rl_repo")

import bass_rust as _bass_rust  # noqa: E402
import concourse.tile as tile  # noqa: E402
from concourse import bacc, mybir  # noqa: E402
from concourse.bass_utils import run_bass_kernel_spmd  # noqa: E402
from concourse.hw_specs import get_activation_tables  # noqa: E402


class _AttnBacc(bacc.Bacc):
    """Bacc whose ACT table-load pass resolves every activation to the
    natural_log_exp_and_others set (it holds Exp, Ln, Copy and Square), so the
    kernel pays exactly one ~2.7us ACT_TABLE_LOAD instead of thrashing between
    the exp-first and ln-first sets on every Ln<->Exp transition."""

    def insert_act_table_loads(self):
        has_activation = any(
            isinstance(i, mybir.InstActivation)
            for b in self.main_func.blocks
            for i in b.instructions
        )
        if not has_activation:
            return
        keep = "natural_log_exp_and_others"
        tables = [
            (name, (fns if name == keep else set()))
            for name, fns in get_activation_tables(self.m.arch).items()
        ]
        _bass_rust.insert_act_table_loads(self, tables)

T, D, H, HD = 2048, 2048, 16, 128
WINDOW = 512
THETA = 10000.0
EPS = 1e-6
NCORES = 8
HPC = H // NCORES            # heads per core
E = HPC * HD                 # per-core projection width (256)
DT = D // 128                # contraction tiles (16)
TCH = 256                    # t-chunk for projections and attention
NCH = T // TCH               # 8
NT = T // 128                # 16
SCALE = float(HD) ** -0.5

F32 = mybir.dt.float32
F32R = mybir.dt.float32r
AF = mybir.ActivationFunctionType

# attention s-tile offsets (m = s_tile - 2c): mask index per m, None = unmasked
MASK_IDX = {1: 0, 0: 1, -3: 2, -4: 3}


def build_nc():
    nc = _AttnBacc(None, target_bir_lowering=False)

    hT_d = nc.dram_tensor("hT", [DT, 128, T], F32, kind="ExternalInput")
    wq_d = nc.dram_tensor("wq", [DT, 128, E], F32, kind="ExternalInput")
    wk_d = nc.dram_tensor("wk", [DT, 128, E], F32, kind="ExternalInput")
    wv_d = nc.dram_tensor("wv", [DT, 128, E], F32, kind="ExternalInput")
    wo_d = nc.dram_tensor("wo", [HPC, 128, D], F32, kind="ExternalInput")
    cos2_d = nc.dram_tensor("cos2", [128, T], F32, kind="ExternalInput")
    sin2_d = nc.dram_tensor("sin2", [128, T], F32, kind="ExternalInput")
    qnw_d = nc.dram_tensor("qnw", [128, 1], F32, kind="ExternalInput")
    knw_d = nc.dram_tensor("knw", [128, 1], F32, kind="ExternalInput")
    masks_d = nc.dram_tensor("masks", [4, 128, TCH], F32, kind="ExternalInput")
    out_d = nc.dram_tensor("out", [T, D], F32, kind="ExternalOutput")

    with tile.TileContext(nc) as tc:
        with (
            tc.tile_pool(name="singles", bufs=1) as singles,
            tc.tile_pool(name="hcp", bufs=3) as hcp,
            tc.tile_pool(name="work", bufs=2) as work,
            tc.tile_pool(name="tabp", bufs=2) as tabp,
            tc.tile_pool(name="expp", bufs=5) as expp,
            tc.tile_pool(name="rowsp", bufs=2) as rowsp,
            tc.tile_pool(name="outst", bufs=3) as outst,
            tc.tile_pool(name="ppb", bufs=2, space="PSUM") as ppb,
            tc.tile_pool(name="pps", bufs=2, space="PSUM") as pps,
        ):
            # ---- constants / weights ----
            wq_sb = singles.tile([128, DT, E], F32R)
            wk_sb = singles.tile([128, DT, E], F32R)
            wv_sb = singles.tile([128, DT, E], F32R)
            nc.sync.dma_start(out=wq_sb, in_=wq_d.rearrange("a p e -> p a e").bitcast(F32R))
            nc.sync.dma_start(out=wk_sb, in_=wk_d.rearrange("a p e -> p a e").bitcast(F32R))
            nc.sync.dma_start(out=wv_sb, in_=wv_d.rearrange("a p e -> p a e").bitcast(F32R))
            wo_sb = singles.tile([128, HPC, D], F32R)
            nc.sync.dma_start(out=wo_sb, in_=wo_d.rearrange("h p n -> p h n").bitcast(F32R))
            # cos/sin tables are loaded per-chunk from DRAM (SBUF is tight)
            qnw_sb = singles.tile([128, 1], F32)
            knw_sb = singles.tile([128, 1], F32)
            nc.sync.dma_start(out=qnw_sb, in_=qnw_d[:, :])
            nc.sync.dma_start(out=knw_sb, in_=knw_d[:, :])
            masks_sb = singles.tile([128, 4, TCH], F32R)
            nc.sync.dma_start(out=masks_sb, in_=masks_d.rearrange("m p c -> p m c").bitcast(F32R))
            ones_f32 = singles.tile([128, 1], F32)
            nc.vector.memset(ones_f32, 1.0)
            ones_col = singles.tile([128, 1], F32R)
            nc.vector.tensor_copy(ones_col[:, :], ones_f32[:, :])
            eps_col = singles.tile([128, 1], F32)
            nc.vector.memset(eps_col, EPS)

            # persistent per-head activations
            q_sb = singles.tile([128, HPC, T], F32R)
            k_sb = singles.tile([128, HPC, T], F32R)
            v_sb = singles.tile([128, NT, E], F32R)
            ctx_sb = singles.tile([128, HPC, T], F32R)

            hT_r = hT_d.rearrange("a p t -> p a t").bitcast(F32R)

            # ===== Phase 1: projections + RMSNorm + RoPE =====
            for c in range(NCH):
                lo = c * TCH
                # hidden chunk split in two half-tiles (SBUF pressure)
                hca = hcp.tile([128, DT // 2, TCH], F32R, tag="hc")
                hcb = hcp.tile([128, DT // 2, TCH], F32R, tag="hc")
                nc.sync.dma_start(out=hca, in_=hT_r[:, 0 : DT // 2, lo : lo + TCH])
                nc.sync.dma_start(out=hcb, in_=hT_r[:, DT // 2 : DT, lo : lo + TCH])

                def hc(d):
                    return hca[:, d, :] if d < DT // 2 else hcb[:, d - DT // 2, :]

                # per-chunk rope tables
                cs_t = tabp.tile([128, TCH], F32, tag="cs")
                sn_t = tabp.tile([128, TCH], F32, tag="sn")
                nc.sync.dma_start(out=cs_t, in_=cos2_d[:, lo : lo + TCH])
                nc.sync.dma_start(out=sn_t, in_=sin2_d[:, lo : lo + TCH])

                # V projection, natural [t, e] layout (2 t-tiles per chunk)
                for t2 in range(2):
                    gt = 2 * c + t2
                    pv = ppb.tile([128, E], F32, tag="v")
                    for d in range(DT):
                        nc.tensor.matmul(
                            pv[:, :],
                            hc(d)[:, t2 * 128 : (t2 + 1) * 128],
                            wv_sb[:, d, :],
                            start=(d == 0),
                            stop=(d == DT - 1),
                        )
                    nc.vector.tensor_copy(v_sb[:, gt, :], pv[:, :])

                # Q/K projections in [e, t] layout + fused norm + rope
                for w_sb, nw_sb, dst in ((wq_sb, qnw_sb, q_sb), (wk_sb, knw_sb, k_sb)):
                    for h in range(HPC):
                        px = ppb.tile([128, TCH], F32, tag="x")
                        for d in range(DT):
                            nc.tensor.matmul(
                                px[:, :],
                                w_sb[:, d, h * HD : (h + 1) * HD],
                                hc(d),
                                start=(d == 0),
                                stop=(d == DT - 1),
                            )
                        # sum over head_dim (partitions) of x^2, via ones-matmul
                        # (ACT Square: a DVE x*x would read PSUM twice, illegal)
                        sq = work.tile([128, TCH], F32R, tag="sq")
                        nc.scalar.activation(sq[:, :], px[:, :], AF.Square)
                        pssq = pps.tile([1, TCH], F32, tag="row")
                        nc.tensor.matmul(pssq[:, :], ones_col[:, :], sq[:, :], start=True, stop=True)
                        # rsqrt(mean + eps) = exp(-0.5 * ln(ssq/HD + eps))
                        lnr = rowsp.tile([1, TCH], F32, tag="lnr")
                        nc.scalar.activation(
                            lnr[:, :], pssq[:, :], AF.Ln, scale=1.0 / HD, bias=eps_col[0:1, 0:1]
                        )
                        rsq = rowsp.tile([1, TCH], F32, tag="rsq")
                        nc.scalar.activation(rsq[:, :], lnr[:, :], AF.Exp, scale=-0.5)
                        # broadcast rsqrt across partitions, then xn = (x * w[p]) * rsqrt[t]
                        bc = work.tile([128, TCH], F32, tag="bc")
                        nc.gpsimd.partition_broadcast(bc[:, :], rsq[:, :])
                        xn = work.tile([128, TCH], F32, tag="xn")
                        nc.vector.scalar_tensor_tensor(
                            xn[:, :], px[:, :], nw_sb[:, 0:1], bc[:, :],
                            op0=mybir.AluOpType.mult, op1=mybir.AluOpType.mult,
                        )
                        # rope: dst[0:64] = x0 c - x1 s ; dst[64:128] = x0 s + x1 c
                        # p1 = xn * [c; c]; p2 holds the partition-swapped sin
                        # products (sin2n rows [0:64) = +s, rows [64:128) = -s):
                        #   p2[64:128] = x0 * (+s), p2[0:64] = x1 * (-s)
                        # so dst = p1 + p2. (DVE TT requires equal input base
                        # partitions; an output-base offset is legal.)
                        p1 = work.tile([128, TCH], F32, tag="p1")
                        p2 = work.tile([128, TCH], F32, tag="p2")
                        nc.vector.tensor_mul(p1[:, :], xn[:, :], cs_t[:, :])
                        nc.vector.tensor_mul(p2[64:128, :], xn[0:64, :], sn_t[0:64, :])
                        nc.vector.tensor_mul(p2[0:64, :], xn[64:128, :], sn_t[64:128, :])
                        nc.vector.tensor_add(dst[:, h, lo : lo + TCH], p1[:, :], p2[:, :])

            # ===== Phase 2: banded attention =====
            for c in range(NCH):
                lo = c * TCH
                for h in range(HPC):
                    ms = [m for m in (-4, -3, -2, -1, 0, 1) if 2 * c + m >= 0]
                    pctx = ppb.tile([128, TCH], F32, tag="v")
                    pden = pps.tile([1, TCH], F32, tag="row")
                    for i, m in enumerate(ms):
                        st = 2 * c + m
                        first, last = (i == 0), (i == len(ms) - 1)
                        psc = ppb.tile([128, TCH], F32, tag="x")
                        nc.tensor.matmul(
                            psc[:, :],
                            k_sb[:, h, st * 128 : (st + 1) * 128],
                            q_sb[:, h, lo : lo + TCH],
                            start=True,
                            stop=True,
                        )
                        et = expp.tile([128, TCH], F32R, tag="exp")
                        nc.scalar.activation(et[:, :], psc[:, :], AF.Exp, scale=SCALE)
                        mi = MASK_IDX.get(m)
                        if mi is not None:
                            nc.vector.tensor_mul(et[:, :], et[:, :], masks_sb[:, mi, :])
                        nc.tensor.matmul(
                            pctx[:, :],
                            v_sb[:, st, h * HD : (h + 1) * HD],
                            et[:, :],
                            start=first,
                            stop=last,
                        )
                        nc.tensor.matmul(
                            pden[:, :], ones_col[:, :], et[:, :], start=first, stop=last
                        )
                    # 1/den = exp(-ln(den)); broadcast across partitions on gpsimd
                    lnd = rowsp.tile([1, TCH], F32, tag="lnr")
                    nc.scalar.activation(lnd[:, :], pden[:, :], AF.Ln)
                    rcp = rowsp.tile([1, TCH], F32, tag="rsq")
                    nc.scalar.activation(rcp[:, :], lnd[:, :], AF.Exp, scale=-1.0)
                    rb = work.tile([128, TCH], F32, tag="bc")
                    nc.gpsimd.partition_broadcast(rb[:, :], rcp[:, :])
                    nc.vector.tensor_mul(ctx_sb[:, h, lo : lo + TCH], pctx[:, :], rb[:, :])

            # ===== Phase 3: out = ctx @ wo (partial over this core's heads) =====
            for t2 in range(NT):
                for j in range(D // 512):
                    po = ppb.tile([128, 512], F32, tag="out")
                    for h in range(HPC):
                        nc.tensor.matmul(
                            po[:, :],
                            ctx_sb[:, h, t2 * 128 : (t2 + 1) * 128],
                            wo_sb[:, h, j * 512 : (j + 1) * 512],
                            start=(h == 0),
                            stop=(h == HPC - 1),
                        )
                    ost = outst.tile([128, 512], F32, tag="ost")
                    if j % 2 == 0:
                        nc.scalar.activation(ost[:, :], po[:, :], AF.Copy)
                    else:
                        nc.vector.tensor_copy(ost[:, :], po[:, :])
                    nc.sync.dma_start(
                        out=out_d[t2 * 128 : (t2 + 1) * 128, j * 512 : (j + 1) * 512],
                        in_=ost[:, :],
                    )

    nc.finalize()
    return nc


def host_prep(hidden_states, wq, wk, wv, wo, q_norm_w, k_norm_w, position_ids):
    """Build the 8 per-core input maps."""
    hidden_states = np.asarray(hidden_states, dtype=np.float32)
    wq = np.asarray(wq, dtype=np.float32)
    wk = np.asarray(wk, dtype=np.float32)
    wv = np.asarray(wv, dtype=np.float32)
    wo = np.asarray(wo, dtype=np.float32)
    q_norm_w = np.asarray(q_norm_w, dtype=np.float32)
    k_norm_w = np.asarray(k_norm_w, dtype=np.float32)
    position_ids = np.asarray(position_ids)

    perm = np.concatenate([np.arange(0, HD, 2), np.arange(1, HD, 2)])

    hTt = np.ascontiguousarray(hidden_states.T).reshape(DT, 128, T)

    inv_freq = (1.0 / (THETA ** (np.arange(0, HD, 2, dtype=np.float32) / HD))).astype(np.float32)
    angles = position_ids.astype(np.float32)[:, None] * inv_freq[None, :]  # [T, 64]
    cosT = np.ascontiguousarray(np.cos(angles).astype(np.float32).T)  # [64, T]
    sinT = np.ascontiguousarray(np.sin(angles).astype(np.float32).T)
    cos2 = np.concatenate([cosT, cosT], axis=0)
    # rows [0:64) = +sin (builds x0*s into out[64:128)), rows [64:128) = -sin
    # (builds -x1*s into out[0:64)) — see the rope comment in build_nc.
    sin2 = np.concatenate([sinT, -sinT], axis=0)

    row = np.arange(128)[:, None]
    col = np.arange(TCH)[None, :]

    def mk(m):
        dd = col - row - 128 * m
        return ((dd >= 0) & (dd < WINDOW)).astype(np.float32)

    masks = np.ascontiguousarray(np.stack([mk(1), mk(0), mk(-3), mk(-4)]))

    wq4 = wq.reshape(D, H, HD)[:, :, perm]
    wk4 = wk.reshape(D, H, HD)[:, :, perm]
    wv4 = wv.reshape(D, H, HD)
    wo4 = wo.reshape(H, HD, D)
    qnw = np.ascontiguousarray(q_norm_w[perm])[:, None]
    knw = np.ascontiguousarray(k_norm_w[perm])[:, None]

    in_maps = []
    for c0 in range(NCORES):
        hs = list(range(HPC * c0, HPC * (c0 + 1)))
        in_maps.append(
            dict(
                hT=hTt,
                wq=np.ascontiguousarray(wq4[:, hs, :]).reshape(DT, 128, E),
                wk=np.ascontiguousarray(wk4[:, hs, :]).reshape(DT, 128, E),
                wv=np.ascontiguousarray(wv4[:, hs, :]).reshape(DT, 128, E),
                wo=np.ascontiguousarray(wo4[hs, :, :]),
                cos2=cos2,
                sin2=sin2,
                qnw=qnw,
                knw=knw,
                masks=masks,
            )
        )
    return in_maps


_NC = None


def kernel(hidden_states, wq, wk, wv, wo, q_norm_w, k_norm_w, position_ids):
    global _NC
    if _NC is None:
        _NC = build_nc()
    in_maps = host_prep(hidden_states, wq, wk, wv, wo, q_norm_w, k_norm_w, position_ids)
    res = run_bass_kernel_spmd(_NC, in_maps, core_ids=list(range(NCORES)))
    acc = np.zeros((T, D), dtype=np.float64)
    for r in res.results:
        acc += r["out"].astype(np.float64)
    return acc.astype(np.float32)


# revision 38
# speedup vs baseline: 5.4536x; 5.4536x over previous
"""Trainium2 Bass kernel for sliding-window attention (nn_AttentionBase).

Reference computation (T=2048, D=2048, H=16, HD=128, WINDOW=512):
  q/k/v = hidden @ wq/wk/wv ; q,k: per-head RMSNorm + RoPE
  scores = q k^T / sqrt(HD), causal sliding-window mask (t-s in [0, 512))
  out = softmax(scores) @ v @ wo

Sharding: tensor-parallel over heads, 2 heads per core on 8 cores; each core
computes a partial out = ctx_heads @ wo_heads; host sums the 8 partials.

Device-side design notes:
- All big matmuls run as float32r (full PE rate at free-dim >= 256).
- hidden is fed pre-transposed (hT), so Q/K come out in [head_dim, t] layout
  directly, K never needs a transpose for scores = K^T_tile.T @ Q.
- The head_dim axis of wq/wk is pre-permuted (even dims | odd dims) so RoPE
  pairs become partition blocks [0:64) and [64:128).
- Softmax runs without max-subtraction (|scaled scores| <= sqrt(HD) ~ 11.3,
  exp stays finite in fp32); the mask is a 0/1 multiply after exp; the
  denominator comes from a ones-column matmul, its reciprocal from
  exp(-ln(den)) on ACT (keeps every ACT op in the exp/ln table set), and is
  partition-broadcast with a K=1 matmul.
"""

import sys

import numpy as np

if "/opt/trn_rl_repo" not in sys.path:
    sys.path.insert(0, "/opt/trn_rl_repo")

import bass_rust as _bass_rust  # noqa: E402
import concourse.tile as tile  # noqa: E402
from concourse import bacc, mybir  # noqa: E402
from concourse.bass_utils import run_bass_kernel_spmd  # noqa: E402
from concourse.hw_specs import get_activation_tables  # noqa: E402


class _AttnBacc(bacc.Bacc):
    """Bacc whose ACT table-load pass resolves every activation to the
    natural_log_exp_and_others set (it holds Exp, Ln, Copy and Square), so the
    kernel pays exactly one ~2.7us ACT_TABLE_LOAD instead of thrashing between
    the exp-first and ln-first sets on every Ln<->Exp transition."""

    def insert_act_table_loads(self):
        has_activation = any(
            isinstance(i, mybir.InstActivation)
            for b in self.main_func.blocks
            for i in b.instructions
        )
        if not has_activation:
            return
        keep = "natural_log_exp_and_others"
        tables = [
            (name, (fns if name == keep else set()))
            for name, fns in get_activation_tables(self.m.arch).items()
        ]
        _bass_rust.insert_act_table_loads(self, tables)

T, D, H, HD = 2048, 2048, 16, 128
WINDOW = 512
THETA = 10000.0
EPS = 1e-6
NCORES = 8
HPC = H // NCORES            # heads per core
E = HPC * HD                 # per-core projection width (256)
DT = D // 128                # contraction tiles (16)
TCH = 256                    # t-chunk for projections and attention
NCH = T // TCH               # 8
NT = T // 128                # 16
SCALE = float(HD) ** -0.5

F32 = mybir.dt.float32
F32R = mybir.dt.float32r
AF = mybir.ActivationFunctionType

# attention s-tile offsets (m = s_tile - 2c): mask index per m, None = unmasked
MASK_IDX = {1: 0, 0: 1, -3: 2, -4: 3}


def build_nc(repeat=1):
    nc = _AttnBacc(None, target_bir_lowering=False)

    hT_d = nc.dram_tensor("hT", [DT, 128, T], F32, kind="ExternalInput")
    wq_d = nc.dram_tensor("wq", [DT, 128, E], F32, kind="ExternalInput")
    wk_d = nc.dram_tensor("wk", [DT, 128, E], F32, kind="ExternalInput")
    wv_d = nc.dram_tensor("wv", [DT, 128, E], F32, kind="ExternalInput")
    wo_d = nc.dram_tensor("wo", [HPC, 128, D], F32, kind="ExternalInput")
    cos2_d = nc.dram_tensor("cos2", [128, T], F32, kind="ExternalInput")
    sin2_d = nc.dram_tensor("sin2", [128, T], F32, kind="ExternalInput")
    qnw_d = nc.dram_tensor("qnw", [128, 1], F32, kind="ExternalInput")
    knw_d = nc.dram_tensor("knw", [128, 1], F32, kind="ExternalInput")
    masks_d = nc.dram_tensor("masks", [2, 128, 2 * TCH], F32, kind="ExternalInput")
    out_d = nc.dram_tensor("out", [T, D], F32, kind="ExternalOutput")

    with tile.TileContext(nc) as tc:
        with (
            tc.tile_pool(name="singles", bufs=1) as singles,
            tc.tile_pool(name="hcp", bufs=3) as hcp,
            tc.tile_pool(name="work", bufs=2) as work,
            tc.tile_pool(name="tabp", bufs=2) as tabp,
            tc.tile_pool(name="expp", bufs=4) as expp,
            tc.tile_pool(name="rowsp", bufs=2) as rowsp,
            tc.tile_pool(name="outst", bufs=2) as outst,
            tc.tile_pool(name="ppx", bufs=2, space="PSUM") as ppx,
            tc.tile_pool(name="ppbig", bufs=4, space="PSUM") as ppbig,
            tc.tile_pool(name="pps", bufs=2, space="PSUM") as pps,
        ):
            # ---- small constants first ----
            qnw_sb = singles.tile([128, 1], F32)
            knw_sb = singles.tile([128, 1], F32)
            nc.sync.dma_start(out=qnw_sb, in_=qnw_d[:, :])
            nc.sync.dma_start(out=knw_sb, in_=knw_d[:, :])
            masks_sb = singles.tile([128, 2, 2 * TCH], F32R)
            ones_f32 = singles.tile([128, 1], F32)
            nc.vector.memset(ones_f32, 1.0)
            ones_col = singles.tile([128, 1], F32R)
            nc.vector.tensor_copy(ones_col[:, :], ones_f32[:, :])
            eps_col = singles.tile([128, 1], F32)
            nc.vector.memset(eps_col, EPS)

            # persistent per-head activations
            q_sb = singles.tile([128, HPC, T], F32R)
            k_sb = singles.tile([128, HPC, T], F32R)
            v_sb = singles.tile([128, NT, E], F32R)
            ctx_sb = singles.tile([128, HPC, T], F32R)

            hT_r = hT_d.rearrange("a p t -> p a t").bitcast(F32R)
            HDT = DT // 2

            wq_sb = singles.tile([128, DT, E], F32R)
            wk_sb = singles.tile([128, DT, E], F32R)
            wv_sb = singles.tile([128, DT, E], F32R)

            # ---- chunk inputs (two half-tiles per chunk) ----
            def load_chunk(c):
                lo = c * TCH
                hca = hcp.tile([128, HDT, TCH], F32R, tag="hc")
                hcb = hcp.tile([128, HDT, TCH], F32R, tag="hc")
                nc.sync.dma_start(out=hca, in_=hT_r[:, 0:HDT, lo : lo + TCH])
                nc.sync.dma_start(out=hcb, in_=hT_r[:, HDT:DT, lo : lo + TCH])
                cs_t = tabp.tile([128, TCH], F32, tag="cs")
                sn_t = tabp.tile([128, TCH], F32, tag="sn")
                nc.sync.dma_start(out=cs_t, in_=cos2_d[:, lo : lo + TCH])
                nc.sync.dma_start(out=sn_t, in_=sin2_d[:, lo : lo + TCH])
                return [hca, hcb], cs_t, sn_t

            chunk0 = load_chunk(0)

            # ---- weights after the chunk-0 stream; masks before wk so
            # attend(0) never waits; wo right after wk ----
            wo_sb = singles.tile([128, HPC, D], F32R)
            wo_r = wo_d.rearrange("h p n -> p h n").bitcast(F32R)
            for wsb, wd in ((wv_sb, wv_d), (wq_sb, wq_d)):
                for sl in (slice(0, HDT), slice(HDT, DT)):
                    nc.sync.dma_start(
                        out=wsb[:, sl, :],
                        in_=wd.rearrange("a p e -> p a e")[:, sl, :].bitcast(F32R),
                    )
            nc.sync.dma_start(out=masks_sb, in_=masks_d.rearrange("m p c -> p m c").bitcast(F32R))
            for sl in (slice(0, HDT), slice(HDT, DT)):
                nc.sync.dma_start(
                    out=wk_sb[:, sl, :],
                    in_=wk_d.rearrange("a p e -> p a e")[:, sl, :].bitcast(F32R),
                )
            nc.sync.dma_start(out=wo_sb[:, 0, :], in_=wo_r[:, 0, :])
            nc.sync.dma_start(out=wo_sb[:, 1, :], in_=wo_r[:, 1, :])

            def project_chunk(c, tiles):
                lo = c * TCH
                hqs, cs_t, sn_t = tiles

                def hc(d):
                    return hqs[d // HDT][:, d % HDT, :]

                # V projection, natural [t, e] layout (2 t-tiles per chunk)
                for t2 in range(2):
                    gt = 2 * c + t2
                    pv = ppx.tile([128, E], F32, tag="x")
                    for d in range(DT):
                        nc.tensor.matmul(
                            pv[:, :],
                            hc(d)[:, t2 * 128 : (t2 + 1) * 128],
                            wv_sb[:, d, :],
                            start=(d == 0),
                            stop=(d == DT - 1),
                        )
                    nc.vector.tensor_copy(v_sb[:, gt, :], pv[:, :])

                # Q/K projections in [e, t] layout + fused norm + rope
                for w_sb, nw_sb, dst in ((wq_sb, qnw_sb, q_sb), (wk_sb, knw_sb, k_sb)):
                    for h in range(HPC):
                        px = ppx.tile([128, TCH], F32, tag="x")
                        for d in range(DT):
                            nc.tensor.matmul(
                                px[:, :],
                                w_sb[:, d, h * HD : (h + 1) * HD],
                                hc(d),
                                start=(d == 0),
                                stop=(d == DT - 1),
                            )
                        # x^2 on ACT (a DVE x*x would read PSUM twice, illegal)
                        sq = work.tile([128, TCH], F32R, tag="sq")
                        nc.scalar.activation(sq[:, :], px[:, :], AF.Square)
                        pssq = pps.tile([1, TCH], F32, tag="row")
                        nc.tensor.matmul(pssq[:, :], ones_col[:, :], sq[:, :], start=True, stop=True)
                        # rsqrt(mean + eps) = exp(-0.5 * ln(ssq/HD + eps))
                        lnr = rowsp.tile([1, TCH], F32, tag="lnr")
                        nc.scalar.activation(
                            lnr[:, :], pssq[:, :], AF.Ln, scale=1.0 / HD, bias=eps_col[0:1, 0:1]
                        )
                        rsq = rowsp.tile([1, TCH], F32, tag="rsq")
                        nc.scalar.activation(rsq[:, :], lnr[:, :], AF.Exp, scale=-0.5)
                        bc = work.tile([128, TCH], F32, tag="bc")
                        nc.gpsimd.partition_broadcast(bc[:, :], rsq[:, :])
                        # xn = (x * w[p]) * rsqrt[t]
                        xn = work.tile([128, TCH], F32, tag="xn")
                        nc.vector.scalar_tensor_tensor(
                            xn[:, :], px[:, :], nw_sb[:, 0:1], bc[:, :],
                            op0=mybir.AluOpType.mult, op1=mybir.AluOpType.mult,
                        )
                        # rope: dst[0:64] = x0 c - x1 s ; dst[64:128] = x0 s + x1 c
                        # p1 = xn * [c; c]; p2 holds the partition-swapped sin
                        # products (sin2n rows [0:64) = +s, rows [64:128) = -s):
                        #   p2[64:128] = x0 * (+s), p2[0:64] = x1 * (-s)
                        # so dst = p1 + p2. (DVE TT requires equal input base
                        # partitions; an output-base offset is legal.)
                        p1 = work.tile([128, TCH], F32, tag="p1")
                        p2 = work.tile([128, TCH], F32, tag="p2")
                        nc.vector.tensor_mul(p1[:, :], xn[:, :], cs_t[:, :])
                        nc.vector.tensor_mul(p2[64:128, :], xn[0:64, :], sn_t[0:64, :])
                        nc.vector.tensor_mul(p2[0:64, :], xn[64:128, :], sn_t[64:128, :])
                        nc.vector.tensor_add(dst[:, h, lo : lo + TCH], p1[:, :], p2[:, :])

            def attend_chunk(c):
                lo = c * TCH
                for h in range(HPC):
                    # s-tile offsets m = st - 2c come in (even, odd) pairs; each
                    # pair shares one [128, 512] psum tile -> one exp, and the
                    # two masked pairs ((-4,-3) and (0,1)) use a combined mask.
                    pairs = [m for m in (-4, -2, 0) if 2 * c + m >= 0]
                    pctx = ppbig.tile([128, TCH], F32, tag="big")
                    pden = pps.tile([1, TCH], F32, tag="row")
                    for i, m in enumerate(pairs):
                        st = 2 * c + m
                        first, last = (i == 0), (i == len(pairs) - 1)
                        psc = ppbig.tile([128, 2 * TCH], F32, tag="big")
                        for half in range(2):
                            nc.tensor.matmul(
                                psc[:, half * TCH : (half + 1) * TCH],
                                k_sb[:, h, (st + half) * 128 : (st + half + 1) * 128],
                                q_sb[:, h, lo : lo + TCH],
                                start=True,
                                stop=True,
                            )
                        et = expp.tile([128, 2 * TCH], F32R, tag="exp")
                        nc.scalar.activation(et[:, :], psc[:, :], AF.Exp, scale=SCALE)
                        mi = {-4: 0, 0: 1}.get(m)
                        if mi is not None:
                            nc.vector.tensor_mul(et[:, :], et[:, :], masks_sb[:, mi, :])
                        for half in range(2):
                            eth = et[:, half * TCH : (half + 1) * TCH]
                            nc.tensor.matmul(
                                pctx[:, :],
                                v_sb[:, st + half, h * HD : (h + 1) * HD],
                                eth,
                                start=(first and half == 0),
                                stop=(last and half == 1),
                            )
                            nc.tensor.matmul(
                                pden[:, :], ones_col[:, :], eth,
                                start=(first and half == 0),
                                stop=(last and half == 1),
                            )
                    # 1/den = exp(-ln(den)); broadcast across partitions on gpsimd
                    lnd = rowsp.tile([1, TCH], F32, tag="lnr")
                    nc.scalar.activation(lnd[:, :], pden[:, :], AF.Ln)
                    rcp = rowsp.tile([1, TCH], F32, tag="rsq")
                    nc.scalar.activation(rcp[:, :], lnd[:, :], AF.Exp, scale=-1.0)
                    rb = work.tile([128, TCH], F32, tag="bc")
                    nc.gpsimd.partition_broadcast(rb[:, :], rcp[:, :])
                    nc.vector.tensor_mul(ctx_sb[:, h, lo : lo + TCH], pctx[:, :], rb[:, :])

            def wo_chunk(c):
                for t2 in (2 * c, 2 * c + 1):
                    for j in range(D // 512):
                        po = ppbig.tile([128, 512], F32, tag="big")
                        for h in range(HPC):
                            nc.tensor.matmul(
                                po[:, :],
                                ctx_sb[:, h, t2 * 128 : (t2 + 1) * 128],
                                wo_sb[:, h, j * 512 : (j + 1) * 512],
                                start=(h == 0),
                                stop=(h == HPC - 1),
                            )
                        ost = outst.tile([128, 512], F32, tag="ost")
                        if j % 2 == 0:
                            nc.scalar.activation(ost[:, :], po[:, :], AF.Copy)
                        else:
                            nc.vector.tensor_copy(ost[:, :], po[:, :])
                        nc.sync.dma_start(
                            out=out_d[t2 * 128 : (t2 + 1) * 128, j * 512 : (j + 1) * 512],
                            in_=ost[:, :],
                        )

            # ===== software-pipelined chunk loop =====
            # attention/WO run one chunk behind the projections: the scores
            # LDWEIGHTS otherwise catches up to k_sb only ~2-4us after the
            # DVE rope writes it, stalling PE once per chunk (and letting HAM
            # re-throttle). The one-chunk lag gives the norm/rope chain a
            # full chunk of slack.
            # (repeat > 1 re-runs the whole computation; used only to measure
            # pure device time as the slope of exec time vs repeat)
            for rep in range(repeat):
                for c in range(NCH):
                    tiles = chunk0 if (c == 0 and rep == 0) else load_chunk(c)
                    project_chunk(c, tiles)
                    attend_chunk(c)
                    wo_chunk(c)

    nc.finalize()
    return nc


def host_prep(hidden_states, wq, wk, wv, wo, q_norm_w, k_norm_w, position_ids):
    """Build the 8 per-core input maps."""
    hidden_states = np.asarray(hidden_states, dtype=np.float32)
    wq = np.asarray(wq, dtype=np.float32)
    wk = np.asarray(wk, dtype=np.float32)
    wv = np.asarray(wv, dtype=np.float32)
    wo = np.asarray(wo, dtype=np.float32)
    q_norm_w = np.asarray(q_norm_w, dtype=np.float32)
    k_norm_w = np.asarray(k_norm_w, dtype=np.float32)
    position_ids = np.asarray(position_ids)

    perm = np.concatenate([np.arange(0, HD, 2), np.arange(1, HD, 2)])

    hTt = np.ascontiguousarray(hidden_states.T).reshape(DT, 128, T)

    inv_freq = (1.0 / (THETA ** (np.arange(0, HD, 2, dtype=np.float32) / HD))).astype(np.float32)
    angles = position_ids.astype(np.float32)[:, None] * inv_freq[None, :]  # [T, 64]
    cosT = np.ascontiguousarray(np.cos(angles).astype(np.float32).T)  # [64, T]
    sinT = np.ascontiguousarray(np.sin(angles).astype(np.float32).T)
    cos2 = np.concatenate([cosT, cosT], axis=0)
    # rows [0:64) = +sin (builds x0*s into out[64:128)), rows [64:128) = -sin
    # (builds -x1*s into out[0:64)) — see the rope comment in build_nc.
    sin2 = np.concatenate([sinT, -sinT], axis=0)

    row = np.arange(128)[:, None]
    col = np.arange(TCH)[None, :]

    def mk(m):
        dd = col - row - 128 * m
        return ((dd >= 0) & (dd < WINDOW)).astype(np.float32)

    masks = np.ascontiguousarray(
        np.stack(
            [
                np.concatenate([mk(-4), mk(-3)], axis=1),
                np.concatenate([mk(0), mk(1)], axis=1),
            ]
        )
    )

    wq4 = wq.reshape(D, H, HD)[:, :, perm]
    wk4 = wk.reshape(D, H, HD)[:, :, perm]
    wv4 = wv.reshape(D, H, HD)
    wo4 = wo.reshape(H, HD, D)
    qnw = np.ascontiguousarray(q_norm_w[perm])[:, None]
    knw = np.ascontiguousarray(k_norm_w[perm])[:, None]

    in_maps = []
    for c0 in range(NCORES):
        hs = list(range(HPC * c0, HPC * (c0 + 1)))
        in_maps.append(
            dict(
                hT=hTt,
                wq=np.ascontiguousarray(wq4[:, hs, :]).reshape(DT, 128, E),
                wk=np.ascontiguousarray(wk4[:, hs, :]).reshape(DT, 128, E),
                wv=np.ascontiguousarray(wv4[:, hs, :]).reshape(DT, 128, E),
                wo=np.ascontiguousarray(wo4[hs, :, :]),
                cos2=cos2,
                sin2=sin2,
                qnw=qnw,
                knw=knw,
                masks=masks,
            )
        )
    return in_maps


_NC = None


def kernel(hidden_states, wq, wk, wv, wo, q_norm_w, k_norm_w, position_ids):
    global _NC
    if _NC is None:
        _NC = build_nc()
    in_maps = host_prep(hidden_states, wq, wk, wv, wo, q_norm_w, k_norm_w, position_ids)
    res = run_bass_kernel_spmd(_NC, in_maps, core_ids=list(range(NCORES)))
    acc = np.zeros((T, D), dtype=np.float64)
    for r in res.results:
        acc += r["out"].astype(np.float64)
    return acc.astype(np.float32)
